# revision 40
# baseline (speedup 1.0000x reference)
"""Multi-head attention (B=4, S=2048, D=1024, H=16, DH=64) on 8 TRN2 cores.

Sharding: core c -> (batch b = c//2, head-group g = c%2 of 8 heads).
Each core computes its batch's attention for its 8 heads plus the partial
W_O projection; the host sums the two partial outputs per batch (the
"all-reduce after W_O" done at unshard time).

Device kernel (per core):
  inputs:  xT (D,S) = x[b].T, wq/wk/wv (D, 512) head-major col slices,
           wo (512, D) row slice
  - QT/KT: per head-pair packed (128, S) = (Wq_pair^T x^T), f32r matmuls
  - V: natural (S-chunk, head*65 cols) bf16 with a ones column per head so
    attnV's extra output row = softmax denominator
  - scoresT (s_k, s_q) per head = KT-slice^T x QT-slice (f32r), exp on
    ScalarE -> bf16 P tiles (no max subtraction: scores ~ N(0,1), fp32-safe)
  - O_aug^T (65, s_q) = V_aug^T @ P^T (bf16); row 64 = denom; normalize via
    reciprocal + partition-broadcast multiply
  - out partial (S, D) accumulated over heads via K=64 f32r matmuls with Wo

All DMA-written SBUF tiles are single-assignment (no slot reuse): DMA
descriptors only support one sync-wait command, so input DMAs may only
carry their queue-FIFO wait.
"""

import sys

if "/opt/trn_rl_repo" not in sys.path:
    sys.path.insert(0, "/opt/trn_rl_repo")

import numpy as np

import concourse.bass as bass
import concourse.tile as tile
from concourse import bacc
from concourse import mybir
from concourse import bass_utils

B, S, D, H, DH = 4, 2048, 1024, 16, 64
HL = 8              # heads per core
NCORES = 8
F32 = mybir.dt.float32
F32R = mybir.dt.float32r
BF16 = mybir.dt.bfloat16
EXP = mybir.ActivationFunctionType.Exp

NDC = D // 128      # 8 d-chunks of 128
NKC = S // 128      # 16 s_k chunks of 128
NSQ = S // 512      # 4 s_q chunks of 512


def _kernel_body(tc):
    nc = tc.nc
    xT = nc.dram_tensor("xT", (D, S), F32R, kind="ExternalInput").ap()
    wq = nc.dram_tensor("wq", (D, HL * DH), F32R, kind="ExternalInput").ap()
    wk = nc.dram_tensor("wk", (D, HL * DH), F32R, kind="ExternalInput").ap()
    wv = nc.dram_tensor("wv", (D, HL * DH), F32R, kind="ExternalInput").ap()
    wo = nc.dram_tensor("wo", (HL * DH, D), F32R, kind="ExternalInput").ap()
    out = nc.dram_tensor("out", (S, D), F32, kind="ExternalOutput").ap()

    with tc.tile_pool(name="persist", bufs=1) as persist:
        # Q^T / K^T packed per head pair: rows 0-63 head 2p, 64-127 head 2p+1
        qt = [persist.tile([128, S], F32R, name=f"qt{p}", tag=f"qt{p}") for p in range(4)]
        kt = [persist.tile([128, S], F32R, name=f"kt{p}", tag=f"kt{p}") for p in range(4)]
        # V natural bf16, 65 cols per head (64 V + 1 ones)
        vv = [persist.tile([128, HL * 65], BF16, name=f"v{sc}", tag=f"v{sc}") for sc in range(NKC)]

        # ---------------- Phase A: projections ----------------
        with tc.tile_pool(name="xtp", bufs=8) as xtp, \
             tc.tile_pool(name="wp", bufs=8) as wp, \
             tc.tile_pool(name="psA", bufs=6, space="PSUM") as psA:
            qs = [nc.sync, nc.scalar, nc.gpsimd]
            xt, wqt, wkt, wvt = [], [], [], []
            for dc in range(NDC):
                sl = slice(dc * 128, (dc + 1) * 128)
                t = xtp.tile([128, S], F32R, name=f"xt{dc}", tag="xt")
                for xc in range(4):
                    qs[(dc + xc) % 3].dma_start(out=t[:, xc * 512:(xc + 1) * 512],
                                                in_=xT[sl, xc * 512:(xc + 1) * 512])
                xt.append(t)
                a = wp.tile([128, HL * DH], F32R, name=f"wq{dc}", tag="wq")
                qs[(dc + 1) % 3].dma_start(out=a, in_=wq[sl, :])
                wqt.append(a)
                b_ = wp.tile([128, HL * DH], F32R, name=f"wk{dc}", tag="wk")
                qs[(dc + 2) % 3].dma_start(out=b_, in_=wk[sl, :])
                wkt.append(b_)
                c_ = wp.tile([128, HL * DH], F32R, name=f"wv{dc}", tag="wv")
                qs[dc % 3].dma_start(out=c_, in_=wv[sl, :])
                wvt.append(c_)
            for sc in range(NKC):
                nc.vector.memset(vv[sc], 1.0)

            # Q^T, K^T per head pair
            for p in range(4):
                csl = slice(p * 128, (p + 1) * 128)
                for sq in range(NSQ):
                    ssl = slice(sq * 512, (sq + 1) * 512)
                    ps = psA.tile([128, 512], F32, name=f"psq_{p}_{sq}", tag="ps")
                    for dc in range(NDC):
                        nc.tensor.matmul(ps, wqt[dc][:, csl], xt[dc][:, ssl],
                                         start=(dc == 0), stop=(dc == NDC - 1))
                    nc.vector.tensor_copy(qt[p][:, ssl], ps)
                    ps2 = psA.tile([128, 512], F32, name=f"psk_{p}_{sq}", tag="ps")
                    for dc in range(NDC):
                        nc.tensor.matmul(ps2, wkt[dc][:, csl], xt[dc][:, ssl],
                                         start=(dc == 0), stop=(dc == NDC - 1))
                    nc.vector.tensor_copy(kt[p][:, ssl], ps2)

            # V natural, all 8 heads at once (N=512)
            for sc in range(NKC):
                ps = psA.tile([128, 512], F32, name=f"psv_{sc}", tag="psv", bufs=2)
                for dc in range(NDC):
                    nc.tensor.matmul(ps, xt[dc][:, sc * 128:(sc + 1) * 128], wvt[dc],
                                     start=(dc == 0), stop=(dc == NDC - 1))
                # scatter 8 heads' (128,64) blocks into stride-65 slots
                vsrc = ps.rearrange("p (h x) -> p h x", x=64)
                vdst = vv[sc].rearrange("p (h x) -> p h x", x=65)[:, :, 0:64]
                nc.vector.tensor_copy(vdst, vsrc)

        # ---------------- Phase B: attention + fused out-projection ----------------
        with tc.tile_pool(name="wop", bufs=1) as wop, \
             tc.tile_pool(name="ptp", bufs=22) as ptp, \
             tc.tile_pool(name="otp", bufs=1) as otp, \
             tc.tile_pool(name="rrp", bufs=4) as rrp, \
             tc.tile_pool(name="brp", bufs=4) as brp, \
             tc.tile_pool(name="stg", bufs=5) as stg, \
             tc.tile_pool(name="psS", bufs=2, space="PSUM") as psS, \
             tc.tile_pool(name="psO", bufs=2, space="PSUM") as psO, \
             tc.tile_pool(name="psF", bufs=2, space="PSUM") as psF:

            # Wo per head pair (128 rows = two heads' dh) for K=128 out-proj
            wo_t = [wop.tile([128, D], F32R, name=f"wo{p}", tag=f"wo{p}") for p in range(4)]
            for p in range(4):
                nc.gpsimd.dma_start(out=wo_t[p], in_=wo[p * 128:(p + 1) * 128, :])

            # s_q processed in chunks of 1024 so exp runs on (128,1024) tiles
            otpairs = {}

            def process_head(q2, h):
                # odd head of each pair first: its O^T needs an extra
                # SBUF->SBUF DMA hop, which then overlaps the even head's
                # compute; the last head per pair writes otpair directly
                otpair = otpairs[q2]
                p, rh = h // 2, h % 2
                rsl = slice(rh * 64, (rh + 1) * 64)
                pts = []
                for kc in range(NKC):
                    ps = psS.tile([128, 1024], F32, name=f"pss_{q2}_{h}_{kc}", tag="pss")
                    for half in range(2):
                        nc.tensor.matmul(ps[:, half * 512:(half + 1) * 512],
                                         kt[p][rsl, kc * 128:(kc + 1) * 128],
                                         qt[p][rsl, q2 * 1024 + half * 512: q2 * 1024 + (half + 1) * 512],
                                         start=True, stop=True)
                    pe = ptp.tile([128, 1024], BF16, name=f"pt_{q2}_{h}_{kc}", tag="pt")
                    nc.scalar.activation(pe, ps, EXP, scale=0.125)
                    pts.append(pe)
                osct = None
                if rh == 1:
                    osct = stg.tile([64, 1024], F32R, name=f"os_{q2}_{h}", tag="os", bufs=2)
                for half in range(2):
                    hsl = slice(half * 512, (half + 1) * 512)
                    po = psO.tile([65, 512], F32, name=f"pso_{q2}_{h}_{half}", tag="pso")
                    for kc in range(NKC):
                        nc.tensor.matmul(po, vv[kc][:, h * 65:(h + 1) * 65], pts[kc][:, hsl],
                                         start=(kc == 0), stop=(kc == NKC - 1))
                    rr = rrp.tile([1, 512], F32, name=f"rr_{q2}_{h}_{half}", tag="rr")
                    nc.vector.reciprocal(rr, po[64:65, :])
                    br = brp.tile([64, 512], F32, name=f"br_{q2}_{h}_{half}", tag="br")
                    nc.gpsimd.partition_broadcast(br, rr)
                    if rh == 0:
                        nc.vector.tensor_mul(otpair[p][0:64, hsl], po[0:64, :], br)
                    else:
                        nc.vector.tensor_mul(osct[:, hsl], po[0:64, :], br)
                        # move this half up to partitions 64-127 right away
                        # (gpsimd queue: keep it off the store queue)
                        nc.gpsimd.dma_start(out=otpair[p][64:128, hsl], in_=osct[:, hsl])

            def outproj_group(q2, q16, dcol):
                # K=128 per pair-stacked O^T tile
                otpair = otpairs[q2]
                pf = psF.tile([128, 512], F32, name=f"psf_{q2}_{q16}_{dcol}", tag="psf")
                for p in range(4):
                    nc.tensor.matmul(pf, otpair[p][:, q16 * 128:(q16 + 1) * 128],
                                     wo_t[p][:, dcol * 512:(dcol + 1) * 512],
                                     start=(p == 0), stop=(p == 3))
                st = stg.tile([128, 512], F32, name=f"st_{q2}_{q16}_{dcol}", tag="st")
                nc.vector.tensor_copy(st, pf)
                nc.sync.dma_start(
                    out=out[q2 * 1024 + q16 * 128: q2 * 1024 + (q16 + 1) * 128,
                            dcol * 512:(dcol + 1) * 512],
                    in_=st)

            ORDER = [1, 0, 3, 2, 5, 4, 7, 6]
            GROUPS = [(q16, dcol) for q16 in range(8) for dcol in range(2)]
            for q2 in range(2):
                # O^T pair-stacked: head 2p on partitions 0-63, head 2p+1 on
                # 64-127 (odd head moved up via SBUF->SBUF DMA)
                otpairs[q2] = [otp.tile([128, 1024], F32R, name=f"otp_{q2}_{p}",
                                        tag=f"otp{p}", bufs=2) for p in range(4)]
                for i, h in enumerate(ORDER):
                    process_head(q2, h)
                    if q2 == 1:
                        # interleave q2=0's out-projection as PE filler while
                        # ACT paces q2=1's exps
                        for g in GROUPS[2 * i: 2 * i + 2]:
                            outproj_group(0, *g)
            for g in GROUPS:
                outproj_group(1, *g)


_NC_CACHE = None


def _get_nc():
    global _NC_CACHE
    if _NC_CACHE is None:
        nc = bacc.Bacc("TRN2", target_bir_lowering=False, debug=False)
        with tile.TileContext(nc) as tc:
            _kernel_body(tc)
        nc.compile()
        _NC_CACHE = nc
    return _NC_CACHE


def _shard_inputs(x, Wq, Wk, Wv, Wo):
    in_maps = []
    for c in range(NCORES):
        b, g = c // 2, c % 2
        xT = np.ascontiguousarray(x[b].T)
        sl = slice(HL * g, HL * (g + 1))
        wq_s = np.ascontiguousarray(Wq[sl].transpose(1, 0, 2).reshape(D, HL * DH))
        wk_s = np.ascontiguousarray(Wk[sl].transpose(1, 0, 2).reshape(D, HL * DH))
        wv_s = np.ascontiguousarray(Wv[sl].transpose(1, 0, 2).reshape(D, HL * DH))
        wo_s = np.ascontiguousarray(Wo[HL * DH * g: HL * DH * (g + 1), :])
        in_maps.append({"xT": xT, "wq": wq_s, "wk": wk_s, "wv": wv_s, "wo": wo_s})
    return in_maps


def kernel(**inputs):
    x = np.asarray(inputs["x"], dtype=np.float32)
    Wq = np.asarray(inputs["Wq"], dtype=np.float32)
    Wk = np.asarray(inputs["Wk"], dtype=np.float32)
    Wv = np.asarray(inputs["Wv"], dtype=np.float32)
    Wo = np.asarray(inputs["Wo"], dtype=np.float32)

    nc = _get_nc()
    in_maps = _shard_inputs(x, Wq, Wk, Wv, Wo)
    res = None
    for attempt in range(3):
        try:
            res = bass_utils.run_bass_kernel_spmd(nc, in_maps, core_ids=list(range(NCORES)))
            break
        except Exception:
            # transient axon/NRT device errors recover on retry
            if attempt == 2:
                raise
            import time
            time.sleep(20)
    outs = [res.results[c]["out"] for c in range(NCORES)]
    full = np.stack([outs[2 * b] + outs[2 * b + 1] for b in range(B)], axis=0)
    return full.astype(np.float32)
